# revision 23
# baseline (speedup 1.0000x reference)
"""Trainium2 Bass kernel for the nn_LSTMCell problem.

Strategy: data-parallel over the batch dim (4096 -> 8 cores x 512), weights
replicated. All on-chip compute happens in "transposed" orientation
(hidden on PSUM partitions, batch on the free dim) so every matmul operand
can be DMA'd in its natural, contiguous layout:

    gate.T[h, b] = sum_k W.T[k, h] * act.T[k, b]
    matmul(out[M=h128, N=b512], lhsT=WT_tile[K=k128, M=h128], rhs=actT[K=k128, N=b512])

Mixed precision: i/f/o-gate matmuls run in fp8(e4m3) with DoubleRowSwInterleave
(2 k-tiles per matmul; measured issue rate is the same 216ns as a 1-k-tile bf16
matmul, so DRS halves PE time); the error-critical cell-candidate gate (ic/hc,
goes through tanh into c1) stays bf16. Operands are pre-scaled (W*256, act*16)
so fp8 values sit in the normal range; the 2^-12 descale is folded into the
gate activation instruction. PSUM accumulation is fp32.

v4 structure (vs the 372.8us v1 baseline):
  - k-major emission with a Delta=2 ramp: the x-only matmuls of m-tiles 0+1
    are emitted interleaved per k-chunk, so each fresh x chunk feeds 8
    matmuls (~260GB/s demand, under wire) instead of 4 (~430GB/s, stalls).
    Steady state pipelines x(m+1) behind h(m-1): 6 of 8 PSUM banks live.
  - weights packed into DRAM slabs in exact consumption order, split into
    x-part/h-part tiles (bounds dependency scope), 4 DMA issues per m-tile.
  - x8/h8 fp8 operand forms derived on-device from the bf16 forms by the
    idle vector engine; a small host-packed fp8 head (x k0:4) removes the
    cast from the first matmul's critical path.
  - queue discipline: sync + scalar are hardware-DGE (fast) and carry all
    ramp-critical bytes; gpsimd's software queue starts slow and only gets
    late-needed bulk (h16, c0, bias, far-ahead prefetches).
  - outputs stored bf16 (og+h1 combined into one DMA per m-tile), upcast
    on host; phase-2 slabs prefetched 2 m-tiles ahead; dummy warmup
    matmuls hold the PE HAM clock gate at full rate through the ramp.
"""

import numpy as np
import ml_dtypes
from contextlib import ExitStack

BF = ml_dtypes.bfloat16
F8 = ml_dtypes.float8_e4m3

N_CORES = 8
P = 128          # partition dim / k-tile size / m-tile size
BATCH = 4096
IN_DIM = 2048
HID = 2048
B = BATCH // N_CORES          # 512, batch per core = matmul free dim
NK = 2048 // P                # 16, k-tiles per weight matrix contraction
MT = HID // P                 # 16, output h-tiles
NPAIR = NK // 2               # 8, fp8 DRS k-tile pairs per matrix

SW = 256.0   # host-side weight scale (all matrices, both dtypes)
SA = 16.0    # host-side activation scale (x/h/c0 and on-device c1)
INV_S = 1.0 / (SW * SA)

WARMUP_MMS = 32  # dummy N=512 matmuls at t0: keep the PE busy (HAM clock
                 # gate warm) while the first real operands stream in


def _build(p, nk, mt, b):
    import concourse.tile as tile
    from concourse import bacc, mybir

    bf16, f32 = mybir.dt.bfloat16, mybir.dt.float32
    f8 = mybir.dt.float8e4
    Sig = mybir.ActivationFunctionType.Sigmoid
    Tanh = mybir.ActivationFunctionType.Tanh
    Copy = mybir.ActivationFunctionType.Copy
    DRS = mybir.MatmulPerfMode.DoubleRowSwInterleave

    nc = bacc.Bacc(
        "TRN2",
        target_bir_lowering=False,
        debug=False,
        num_devices=N_CORES,
    )

    xT16 = nc.dram_tensor("xT16", [p, nk, b], bf16, kind="ExternalInput").ap()
    xT8h = nc.dram_tensor("xT8h", [p, 4, b], f8, kind="ExternalInput").ap()
    hT16 = nc.dram_tensor("hT16", [p, nk, b], bf16, kind="ExternalInput").ap()
    cT8 = nc.dram_tensor("cT8", [p, nk, b], f8, kind="ExternalInput").ap()
    c0T = nc.dram_tensor("c0T", [p, mt, b], f32, kind="ExternalInput").ap()
    bias = nc.dram_tensor("bias", [p, mt, 4], f32, kind="ExternalInput").ap()

    # phase-1 fp8 slab: per m-tile, 40 DRS pair-columns in consumption order:
    #   x-part cols 0..15  = [ii_j, if_j] for j in 0..7 (interleaved)
    #   h-part cols 16..39 = [hi_j, hf_j, cf_j] for j in 0..7
    wp1a = nc.dram_tensor("wp1a", [mt, p, 40, 2 * p], f8,
                          kind="ExternalInput").ap()
    # phase-1 bf16 slab: x-part cols 0..15 = ic k-tiles, h-part 16..31 = hc
    wp1b = nc.dram_tensor("wp1b", [mt, p, 32, p], bf16,
                          kind="ExternalInput").ap()
    # phase-2 fp8 slab: cols = [io_j, ho_j, co_j] for j in 0..7
    wp2a = nc.dram_tensor("wp2a", [mt, p, 24, 2 * p], f8,
                          kind="ExternalInput").ap()

    ohT = nc.dram_tensor("ohT", [p, mt, 2, b], bf16, kind="ExternalOutput").ap()
    c1T = nc.dram_tensor("c1T", [p, mt, b], bf16, kind="ExternalOutput").ap()

    with tile.TileContext(nc) as tc, ExitStack() as ctx:
        acts = ctx.enter_context(tc.tile_pool(name="acts", bufs=1))
        wpool = ctx.enter_context(tc.tile_pool(name="w", bufs=3))
        cpool = ctx.enter_context(tc.tile_pool(name="c0", bufs=3))
        tpool = ctx.enter_context(tc.tile_pool(name="temps", bufs=2))
        ppool = ctx.enter_context(tc.tile_pool(name="psum", bufs=8, space="PSUM"))

        # ---- resident activation tensors ----
        x16 = acts.tile([p, nk, b], bf16, tag="x16")
        h16 = acts.tile([p, nk, b], bf16, tag="h16")
        x8 = acts.tile([p, nk, b], f8, tag="x8")
        h8 = acts.tile([p, nk, b], f8, tag="h8")
        c8 = acts.tile([p, nk, b], f8, tag="c8")
        c1b = acts.tile([p, mt, b], bf16, tag="c1b")     # new cell state
        c18 = acts.tile([p, mt, b], f8, tag="c18")       # c1*SA in fp8 (co rhs)
        bias_sb = acts.tile([p, mt, 4], f32, tag="bias")

        # ---- HAM warmup: tiny matmuls keep PE busy from t0 so the clock
        # gate is already releasing by the time real data lands ----
        if WARMUP_MMS:
            warm = tpool.tile([p, b], bf16, tag="warm", bufs=1)
            nc.gpsimd.memset(warm[:], 0.0)
            ps_w = ppool.tile([p, b], f32, tag="ps", name="ps_warm")
            for _ in range(WARMUP_MMS):
                nc.tensor.matmul(ps_w[:], lhsT=warm[:, 0:p], rhs=warm[:],
                                 start=True, stop=True)

        # ---- weight slab tiles ----
        ax, ah, bx, bh, slab2 = {}, {}, {}, {}, {}

        def alloc_x(m):
            ax[m] = wpool.tile([p, 16, 2 * p], f8, tag="p1ax", name=f"ax{m}")
            bx[m] = wpool.tile([p, 16, p], bf16, tag="p1bx", name=f"bx{m}")

        def alloc_h(m):
            ah[m] = wpool.tile([p, 24, 2 * p], f8, tag="p1ah", name=f"ah{m}")
            bh[m] = wpool.tile([p, 16, p], bf16, tag="p1bh", name=f"bh{m}")

        def issue_x(m, eng, chunks=((0, 16),)):
            for lo, hi in chunks:
                eng.dma_start(ax[m][:, lo:hi], wp1a[m, :, lo:hi])
                eng.dma_start(bx[m][:, lo:hi], wp1b[m, :, lo:hi])

        def issue_h(m, eng, split=False):
            if split:
                eng.dma_start(ah[m][:, 0:12], wp1a[m, :, 16:28])
                eng.dma_start(bh[m][:, 0:8], wp1b[m, :, 16:24])
                eng.dma_start(ah[m][:, 12:24], wp1a[m, :, 28:40])
                eng.dma_start(bh[m][:, 8:16], wp1b[m, :, 24:32])
            else:
                eng.dma_start(ah[m][:], wp1a[m, :, 16:40])
                eng.dma_start(bh[m][:], wp1b[m, :, 16:32])

        c0_t = {}

        def issue_c0(m, eng):
            c0_t[m] = cpool.tile([p, b], f32, tag="c0", name=f"c0_{m}")
            eng.dma_start(c0_t[m][:], c0T[:, m, :])

        def alloc_p2(m):
            slab2[m] = wpool.tile([p, 24, 2 * p], f8, tag="p2a",
                                  name=f"p2a_{m}")

        def issue_p2(m, eng):
            eng.dma_start(slab2[m][:], wp2a[m])

        # ---- preload ----
        # Per-transfer DMA throughput is low until many transfers are in
        # flight per queue, so the x-stream (the ramp critical path) is
        # interleaved across BOTH fast hardware queues in small chunks,
        # each queue's order matching consumption order.
        alloc_x(0)
        alloc_x(1)
        alloc_x(2)
        alloc_h(0)
        alloc_h(1)
        alloc_h(2)
        nc.sync.dma_start(x8[:, 0:2, :], xT8h[:, 0:2, :])
        issue_x(0, nc.sync, ((0, 2),))
        issue_x(1, nc.sync, ((0, 2),))
        nc.scalar.dma_start(x16[:, 0:2, :], xT16[:, 0:2, :])
        nc.sync.dma_start(x8[:, 2:4, :], xT8h[:, 2:4, :])
        nc.scalar.dma_start(ax[0][:, 2:8], wp1a[0, :, 2:8])
        nc.scalar.dma_start(bx[0][:, 2:8], wp1b[0, :, 2:8])
        nc.sync.dma_start(x16[:, 2:4, :], xT16[:, 2:4, :])
        nc.scalar.dma_start(ax[1][:, 2:8], wp1a[1, :, 2:8])
        nc.scalar.dma_start(bx[1][:, 2:8], wp1b[1, :, 2:8])
        nc.scalar.dma_start(x16[:, 4:6, :], xT16[:, 4:6, :])
        nc.sync.dma_start(x16[:, 6:8, :], xT16[:, 6:8, :])
        nc.scalar.dma_start(ax[0][:, 8:16], wp1a[0, :, 8:16])
        nc.scalar.dma_start(bx[0][:, 8:16], wp1b[0, :, 8:16])
        nc.sync.dma_start(x16[:, 10:12, :], xT16[:, 10:12, :])
        nc.scalar.dma_start(x16[:, 8:10, :], xT16[:, 8:10, :])
        nc.scalar.dma_start(ax[1][:, 8:16], wp1a[1, :, 8:16])
        nc.scalar.dma_start(bx[1][:, 8:16], wp1b[1, :, 8:16])
        nc.sync.dma_start(c8[:, 0:8, :], cT8[:, 0:8, :])
        nc.sync.dma_start(c8[:, 8:16, :], cT8[:, 8:16, :])
        issue_h(0, nc.scalar, split=True)
        issue_h(1, nc.scalar)
        issue_x(2, nc.sync)
        issue_h(2, nc.sync)
        # gpsimd (slow software queue): x16 tail (needed ~15us in), then
        # late-needed bulk
        nc.gpsimd.dma_start(x16[:, 12:14, :], xT16[:, 12:14, :])
        nc.gpsimd.dma_start(x16[:, 14:16, :], xT16[:, 14:16, :])
        for lo, hi in ((0, 4), (4, 8), (8, 12), (12, 16)):
            nc.gpsimd.dma_start(h16[:, lo:hi, :], hT16[:, lo:hi, :])
        issue_c0(0, nc.gpsimd)
        issue_c0(1, nc.gpsimd)
        nc.gpsimd.dma_start(bias_sb[:], bias[:])
        # derive the remaining fp8 forms on the vector engine
        for lo, hi in ((4, 6), (6, 8), (8, 10), (10, 12), (12, 14), (14, 16)):
            nc.vector.tensor_copy(out=x8[:, lo:hi, :], in_=x16[:, lo:hi, :])
        for lo, hi in ((0, 4), (4, 8), (8, 12), (12, 16)):
            nc.vector.tensor_copy(out=h8[:, lo:hi, :], in_=h16[:, lo:hi, :])

        # ---- phase 1 ----
        TOT = {"i": 2 * NPAIR, "f": 3 * NPAIR, "g": 2 * nk}
        rot = (nc.scalar, nc.sync, nc.gpsimd)
        ps = {}
        idx = {}

        def mm(gate, m, lhsT, rhs, perf_mode=None):
            st = idx[(gate, m)] == 0
            sp = idx[(gate, m)] == TOT[gate] - 1
            nc.tensor.matmul(ps[(gate, m)][:], lhsT=lhsT, rhs=rhs,
                             start=st, stop=sp, perf_mode=perf_mode)
            idx[(gate, m)] += 1

        def open_ps(m):
            for g in ("i", "f", "g"):
                ps[(g, m)] = ppool.tile([p, b], f32, tag="ps",
                                        name=f"ps_{g}{m}")
                idx[(g, m)] = 0

        def emit_xj(m, j):
            mm("i", m, ax[m][:, 2 * j], x8[:, 2 * j:2 * j + 2], DRS)
            mm("f", m, ax[m][:, 2 * j + 1], x8[:, 2 * j:2 * j + 2], DRS)
            mm("g", m, bx[m][:, 2 * j], x16[:, 2 * j])
            mm("g", m, bx[m][:, 2 * j + 1], x16[:, 2 * j + 1])

        def emit_h(m):
            for j in range(NPAIR):
                mm("i", m, ah[m][:, 3 * j], h8[:, 2 * j:2 * j + 2], DRS)
                mm("f", m, ah[m][:, 3 * j + 1], h8[:, 2 * j:2 * j + 2], DRS)
                mm("f", m, ah[m][:, 3 * j + 2], c8[:, 2 * j:2 * j + 2], DRS)
                mm("g", m, bh[m][:, 2 * j], h16[:, 2 * j])
                mm("g", m, bh[m][:, 2 * j + 1], h16[:, 2 * j + 1])

        def acts_for(m):
            i_act = tpool.tile([p, b], f32, tag="i_act")
            nc.scalar.activation(i_act[:], ps[("i", m)][:], Sig,
                                 bias=bias_sb[:, m, 0:1], scale=INV_S)
            f_act = tpool.tile([p, b], f32, tag="f_act")
            nc.scalar.activation(f_act[:], ps[("f", m)][:], Sig,
                                 bias=bias_sb[:, m, 1:2], scale=INV_S)
            g_act = tpool.tile([p, b], f32, tag="g_act")
            nc.scalar.activation(g_act[:], ps[("g", m)][:], Tanh,
                                 bias=bias_sb[:, m, 2:3], scale=INV_S)
            t1 = tpool.tile([p, b], f32, tag="t1")
            nc.vector.tensor_mul(t1[:], f_act[:], c0_t[m][:])
            nc.vector.tensor_mul(i_act[:], i_act[:], g_act[:])
            c1_m = c1b[:, m, :]
            nc.vector.tensor_add(c1_m, t1[:], i_act[:])
            nc.scalar.activation(c18[:, m, :], c1_m, Copy, scale=SA)
            nc.sync.dma_start(c1T[:, m, :], c1_m)

        # Delta=2 ramp block: x-parts of m0+m1 interleaved per k-chunk.
        # A few dependency-free dummy matmuls between k-chunks absorb the
        # DMA catch-up time so the PE never idles long enough for the HAM
        # clock gate to re-throttle mid-ramp.
        PAD = (4, 4, 3, 3, 2, 2, 1, 0)
        open_ps(0)
        open_ps(1)
        for j in range(NPAIR):
            emit_xj(0, j)
            emit_xj(1, j)
            for _ in range(PAD[j] if WARMUP_MMS else 0):
                nc.tensor.matmul(ps_w[:], lhsT=warm[:, 0:p], rhs=warm[:],
                                 start=True, stop=True)

        # steady iterations: h(k-1) + acts, then x(k+1)
        for k in range(1, mt + 1):
            if k + 2 < mt:
                alloc_x(k + 2)
                issue_x(k + 2, rot[k % 3])
            if 3 <= k + 1 < mt:
                alloc_h(k + 1)
                issue_h(k + 1, rot[k % 3])
            if k + 1 < mt:
                issue_c0(k + 1, nc.gpsimd)
            if k == mt - 1:
                alloc_p2(0); issue_p2(0, nc.scalar)
                alloc_p2(1); issue_p2(1, nc.scalar)
            emit_h(k - 1)
            acts_for(k - 1)
            if k + 1 < mt:
                open_ps(k + 1)
                for j in range(NPAIR):
                    emit_xj(k + 1, j)

        # ---- phase 2: o gate + h1 ----
        for m in range(mt):
            if m + 2 < mt:
                alloc_p2(m + 2)
                issue_p2(m + 2, nc.sync if m % 2 else nc.scalar)
            ps_o = ppool.tile([p, b], f32, tag="ps", name=f"ps_o{m}")
            # tanh(c1) first: scalar computes it while the matmuls run
            tc1 = tpool.tile([p, b], bf16, tag="tc1")
            nc.scalar.activation(tc1[:], c1b[:, m, :], Tanh)
            for j in range(NPAIR):
                st = j == 0
                sp = j == NPAIR - 1
                nc.tensor.matmul(ps_o[:], lhsT=slab2[m][:, 3 * j],
                                 rhs=x8[:, 2 * j:2 * j + 2],
                                 start=st, stop=False, perf_mode=DRS)
                nc.tensor.matmul(ps_o[:], lhsT=slab2[m][:, 3 * j + 1],
                                 rhs=h8[:, 2 * j:2 * j + 2],
                                 start=False, stop=False, perf_mode=DRS)
                nc.tensor.matmul(ps_o[:], lhsT=slab2[m][:, 3 * j + 2],
                                 rhs=c18[:, 2 * j:2 * j + 2, :],
                                 start=False, stop=sp, perf_mode=DRS)

            oh = tpool.tile([p, 2, b], bf16, tag="oh")
            nc.scalar.activation(oh[:, 0, :], ps_o[:], Sig,
                                 bias=bias_sb[:, m, 3:4], scale=INV_S)
            nc.vector.tensor_mul(oh[:, 1, :], oh[:, 0, :], tc1[:])
            nc.sync.dma_start(ohT[:, m], oh[:])

    nc.compile()
    return nc


_NC = None


def _get_nc():
    global _NC
    if _NC is None:
        _NC = _build(P, NK, MT, B)
    return _NC


# ---------------- host-side packing ----------------

def _pack_actT(a, dtype, scale=1.0):
    """(b, d) -> (128, d//128, b) with [ki, ko, b] = a[b, ko*128+ki]."""
    b, d = a.shape
    at = np.ascontiguousarray(a.T.reshape(d // P, P, b).transpose(1, 0, 2))
    if scale != 1.0:
        at = np.clip(at * scale, -240.0, 240.0)
    return at.astype(dtype, copy=False)


def _pack_w(W, dtype, scale):
    """(H, K) -> (H//128, 128, 16, 128) with [mt, ki, ko, m] = W[mt*128+m, ko*128+ki]."""
    H, K = W.shape
    r = (W * scale).reshape(H // P, P, K // P, P).transpose(0, 3, 2, 1)
    return np.ascontiguousarray(r).astype(dtype)


def _pack_w8_swi(W, scale):
    """fp8 weights in DoubleRowSwInterleave layout: per k-tile pair (A, B),
    each SBUF row holds [A127, B127, A126, B126, ..., A0, B0].
    (H, K) -> (H//128, 128, 8, 256)."""
    r = _pack_w(W, F8, scale)                    # [mt, ki, kt, m]
    A = r[:, :, 0::2, ::-1]                      # [mt, ki, pair, m-reversed]
    Bm = r[:, :, 1::2, ::-1]
    return np.ascontiguousarray(
        np.stack([A, Bm], axis=-1).reshape(A.shape[0], P, A.shape[2], 2 * P))


def _interleave(mats):
    """list of [mt, p, n, w] -> [mt, p, n*len, w] with columns round-robin."""
    s = np.stack(mats, axis=3)                   # [mt, p, n, L, w]
    return np.ascontiguousarray(
        s.reshape(s.shape[0], s.shape[1], -1, s.shape[4]))


def _unpack_out(o):
    """(128, mt, b) [p, m, b] -> (b, mt*128) fp32."""
    p, m, b = o.shape
    return np.ascontiguousarray(
        o.transpose(2, 1, 0).reshape(b, m * p).astype(np.float32))


def kernel(x, h0, c0,
           W_ii, b_ii, W_hi, b_hi, W_if_, b_if_, W_hf, b_hf, W_cf, b_cf,
           W_ic, b_ic, W_hc, b_hc, W_io, b_io, W_ho, b_ho, W_co, b_co,
           _trace=False):
    from concourse.bass_utils import run_bass_kernel_spmd

    nc = _get_nc()

    x = np.asarray(x, dtype=np.float32)
    h0 = np.asarray(h0, dtype=np.float32)
    c0 = np.asarray(c0, dtype=np.float32)
    Ws = {n: np.asarray(a, dtype=np.float32)
          for n, a in (("ii", W_ii), ("hi", W_hi), ("if_", W_if_),
                       ("hf", W_hf), ("cf", W_cf), ("ic", W_ic),
                       ("hc", W_hc), ("io", W_io), ("ho", W_ho),
                       ("co", W_co))}
    (b_ii, b_hi, b_if_, b_hf, b_cf, b_ic, b_hc, b_io, b_ho, b_co) = [
        np.asarray(a, dtype=np.float32)
        for a in (b_ii, b_hi, b_if_, b_hf, b_cf, b_ic, b_hc, b_io, b_ho, b_co)
    ]

    # combined per-gate biases, packed [p, mt, gate]
    bias = np.stack(
        [
            (b_ii + b_hi).reshape(MT, P).T,
            (b_if_ + b_hf + b_cf).reshape(MT, P).T,
            (b_ic + b_hc).reshape(MT, P).T,
            (b_io + b_ho + b_co).reshape(MT, P).T,
        ],
        axis=2,
    ).astype(np.float32)

    w8 = {n: _pack_w8_swi(Ws[n], SW)
          for n in ("ii", "if_", "hi", "hf", "cf", "io", "ho", "co")}
    wp1a = np.concatenate(
        [_interleave([w8["ii"], w8["if_"]]),
         _interleave([w8["hi"], w8["hf"], w8["cf"]])], axis=2)
    wp1b = np.concatenate(
        [_pack_w(Ws["ic"], BF, SW), _pack_w(Ws["hc"], BF, SW)], axis=2)
    wp2a = _interleave([w8["io"], w8["ho"], w8["co"]])

    in_maps = []
    for core in range(N_CORES):
        s = slice(core * B, (core + 1) * B)
        xT16 = _pack_actT(x[s], BF, SA)
        in_maps.append({
            "xT16": xT16,
            # fp8 head for k-tiles 0..3: first matmuls skip the on-device
            # cast; matches the device cast (bf16 -> fp8 double rounding)
            "xT8h": np.ascontiguousarray(xT16[:, 0:4, :]).astype(F8),
            "hT16": _pack_actT(h0[s], BF, SA),
            "cT8": _pack_actT(c0[s], F8, SA),
            "c0T": _pack_actT(c0[s], np.float32),
            "bias": bias,
            "wp1a": wp1a, "wp1b": wp1b, "wp2a": wp2a,
        })

    res = run_bass_kernel_spmd(nc, in_maps, list(range(N_CORES)), trace=_trace)

    o_g = np.empty((BATCH, HID), np.float32)
    h1 = np.empty((BATCH, HID), np.float32)
    c1 = np.empty((BATCH, HID), np.float32)
    for core in range(N_CORES):
        s = slice(core * B, (core + 1) * B)
        oh = res.results[core]["ohT"]
        o_g[s] = _unpack_out(oh[:, :, 0, :])
        h1[s] = _unpack_out(oh[:, :, 1, :])
        c1[s] = _unpack_out(res.results[core]["c1T"])
    out = (o_g, h1, c1)
    if _trace:
        return out, res
    return out


# revision 26
# speedup vs baseline: 1.0030x; 1.0030x over previous
"""Trainium2 Bass kernel for the nn_LSTMCell problem.

Strategy: data-parallel over the batch dim (4096 -> 8 cores x 512), weights
replicated. All on-chip compute happens in "transposed" orientation
(hidden on PSUM partitions, batch on the free dim) so every matmul operand
can be DMA'd in its natural, contiguous layout:

    gate.T[h, b] = sum_k W.T[k, h] * act.T[k, b]
    matmul(out[M=h128, N=b512], lhsT=WT_tile[K=k128, M=h128], rhs=actT[K=k128, N=b512])

Mixed precision: i/f/o-gate matmuls run in fp8(e4m3) with DoubleRowSwInterleave
(2 k-tiles per matmul; measured issue rate is the same 216ns as a 1-k-tile bf16
matmul, so DRS halves PE time); the error-critical cell-candidate gate (ic/hc,
goes through tanh into c1) stays bf16. Operands are pre-scaled (W*256, act*16)
so fp8 values sit in the normal range; the 2^-12 descale is folded into the
gate activation instruction. PSUM accumulation is fp32.

v4 structure (vs the 372.8us v1 baseline):
  - k-major emission with a Delta=2 ramp: the x-only matmuls of m-tiles 0+1
    are emitted interleaved per k-chunk, so each fresh x chunk feeds 8
    matmuls (~260GB/s demand, under wire) instead of 4 (~430GB/s, stalls).
    Steady state pipelines x(m+1) behind h(m-1): 6 of 8 PSUM banks live.
  - weights packed into DRAM slabs in exact consumption order, split into
    x-part/h-part tiles (bounds dependency scope), 4 DMA issues per m-tile.
  - x8/h8 fp8 operand forms derived on-device from the bf16 forms by the
    idle vector engine; a small host-packed fp8 head (x k0:4) removes the
    cast from the first matmul's critical path.
  - queue discipline: sync + scalar are hardware-DGE (fast) and carry all
    ramp-critical bytes; gpsimd's software queue starts slow and only gets
    late-needed bulk (h16, c0, bias, far-ahead prefetches).
  - outputs stored bf16 (og+h1 combined into one DMA per m-tile), upcast
    on host; phase-2 slabs prefetched 2 m-tiles ahead; dummy warmup
    matmuls hold the PE HAM clock gate at full rate through the ramp.
"""

import numpy as np
import ml_dtypes
from contextlib import ExitStack

BF = ml_dtypes.bfloat16
F8 = ml_dtypes.float8_e4m3

N_CORES = 8
P = 128          # partition dim / k-tile size / m-tile size
BATCH = 4096
IN_DIM = 2048
HID = 2048
B = BATCH // N_CORES          # 512, batch per core = matmul free dim
NK = 2048 // P                # 16, k-tiles per weight matrix contraction
MT = HID // P                 # 16, output h-tiles
NPAIR = NK // 2               # 8, fp8 DRS k-tile pairs per matrix

SW = 256.0   # host-side weight scale (all matrices, both dtypes)
SA = 16.0    # host-side activation scale (x/h/c0 and on-device c1)
INV_S = 1.0 / (SW * SA)

WARMUP_MMS = 40  # dummy N=512 matmuls at t0: keep the PE busy (HAM clock
                 # gate warm) while the first real operands stream in


def _build(p, nk, mt, b):
    import concourse.tile as tile
    from concourse import bacc, mybir

    bf16, f32 = mybir.dt.bfloat16, mybir.dt.float32
    f8 = mybir.dt.float8e4
    Sig = mybir.ActivationFunctionType.Sigmoid
    Tanh = mybir.ActivationFunctionType.Tanh
    Copy = mybir.ActivationFunctionType.Copy
    DRS = mybir.MatmulPerfMode.DoubleRowSwInterleave

    nc = bacc.Bacc(
        "TRN2",
        target_bir_lowering=False,
        debug=False,
        num_devices=N_CORES,
    )

    xT16 = nc.dram_tensor("xT16", [p, nk, b], bf16, kind="ExternalInput").ap()
    xT8h = nc.dram_tensor("xT8h", [p, 4, b], f8, kind="ExternalInput").ap()
    hT16 = nc.dram_tensor("hT16", [p, nk, b], bf16, kind="ExternalInput").ap()
    cT8 = nc.dram_tensor("cT8", [p, nk, b], f8, kind="ExternalInput").ap()
    c0T = nc.dram_tensor("c0T", [p, mt, b], f32, kind="ExternalInput").ap()
    bias = nc.dram_tensor("bias", [p, mt, 4], f32, kind="ExternalInput").ap()

    # phase-1 fp8 slab: per m-tile, 40 DRS pair-columns in consumption order:
    #   x-part cols 0..15  = [ii_j, if_j] for j in 0..7 (interleaved)
    #   h-part cols 16..39 = [hi_j, hf_j, cf_j] for j in 0..7
    wp1a = nc.dram_tensor("wp1a", [mt, p, 40, 2 * p], f8,
                          kind="ExternalInput").ap()
    # phase-1 bf16 slab: x-part cols 0..15 = ic k-tiles, h-part 16..31 = hc
    wp1b = nc.dram_tensor("wp1b", [mt, p, 32, p], bf16,
                          kind="ExternalInput").ap()
    # phase-2 fp8 slab: cols = [io_j, ho_j, co_j] for j in 0..7
    wp2a = nc.dram_tensor("wp2a", [mt, p, 24, 2 * p], f8,
                          kind="ExternalInput").ap()

    ohT = nc.dram_tensor("ohT", [p, mt, 2, b], bf16, kind="ExternalOutput").ap()
    c1T = nc.dram_tensor("c1T", [p, mt, b], bf16, kind="ExternalOutput").ap()

    with tile.TileContext(nc) as tc, ExitStack() as ctx:
        acts = ctx.enter_context(tc.tile_pool(name="acts", bufs=1))
        wpool = ctx.enter_context(tc.tile_pool(name="w", bufs=3))
        cpool = ctx.enter_context(tc.tile_pool(name="c0", bufs=3))
        tpool = ctx.enter_context(tc.tile_pool(name="temps", bufs=2))
        ppool = ctx.enter_context(tc.tile_pool(name="psum", bufs=8, space="PSUM"))

        # ---- resident activation tensors ----
        x16 = acts.tile([p, nk, b], bf16, tag="x16")
        h16 = acts.tile([p, nk, b], bf16, tag="h16")
        x8 = acts.tile([p, nk, b], f8, tag="x8")
        h8 = acts.tile([p, nk, b], f8, tag="h8")
        c8 = acts.tile([p, nk, b], f8, tag="c8")
        c1b = acts.tile([p, mt, b], bf16, tag="c1b")     # new cell state
        c18 = acts.tile([p, mt, b], f8, tag="c18")       # c1*SA in fp8 (co rhs)
        bias_sb = acts.tile([p, mt, 4], f32, tag="bias")

        # ---- HAM warmup: tiny matmuls keep PE busy from t0 so the clock
        # gate is already releasing by the time real data lands ----
        if WARMUP_MMS:
            warm = tpool.tile([p, b], bf16, tag="warm", bufs=1)
            nc.gpsimd.memset(warm[:], 0.0)
            ps_w = ppool.tile([p, b], f32, tag="ps", name="ps_warm")
            for _ in range(WARMUP_MMS):
                nc.tensor.matmul(ps_w[:], lhsT=warm[:, 0:p], rhs=warm[:],
                                 start=True, stop=True)

        # ---- weight slab tiles ----
        ax, ah, bx, bh, slab2 = {}, {}, {}, {}, {}

        def alloc_x(m):
            ax[m] = wpool.tile([p, 16, 2 * p], f8, tag="p1ax", name=f"ax{m}")
            bx[m] = wpool.tile([p, 16, p], bf16, tag="p1bx", name=f"bx{m}")

        def alloc_h(m):
            ah[m] = wpool.tile([p, 24, 2 * p], f8, tag="p1ah", name=f"ah{m}")
            bh[m] = wpool.tile([p, 16, p], bf16, tag="p1bh", name=f"bh{m}")

        def issue_x(m, eng, chunks=((0, 16),)):
            for lo, hi in chunks:
                eng.dma_start(ax[m][:, lo:hi], wp1a[m, :, lo:hi])
                eng.dma_start(bx[m][:, lo:hi], wp1b[m, :, lo:hi])

        def issue_h(m, eng, split=False):
            if split:
                eng.dma_start(ah[m][:, 0:12], wp1a[m, :, 16:28])
                eng.dma_start(bh[m][:, 0:8], wp1b[m, :, 16:24])
                eng.dma_start(ah[m][:, 12:24], wp1a[m, :, 28:40])
                eng.dma_start(bh[m][:, 8:16], wp1b[m, :, 24:32])
            else:
                eng.dma_start(ah[m][:], wp1a[m, :, 16:40])
                eng.dma_start(bh[m][:], wp1b[m, :, 16:32])

        c0_t = {}

        def issue_c0(m, eng):
            c0_t[m] = cpool.tile([p, b], f32, tag="c0", name=f"c0_{m}")
            eng.dma_start(c0_t[m][:], c0T[:, m, :])

        def alloc_p2(m):
            slab2[m] = wpool.tile([p, 24, 2 * p], f8, tag="p2a",
                                  name=f"p2a_{m}")

        def issue_p2(m, eng):
            eng.dma_start(slab2[m][:], wp2a[m])

        # ---- preload ----
        # Per-transfer DMA throughput is low until many transfers are in
        # flight per queue, so the x-stream (the ramp critical path) is
        # interleaved across BOTH fast hardware queues in small chunks,
        # each queue's order matching consumption order.
        alloc_x(0)
        alloc_x(1)
        alloc_x(2)
        alloc_h(0)
        alloc_h(1)
        alloc_h(2)
        nc.sync.dma_start(x8[:, 0:2, :], xT8h[:, 0:2, :])
        issue_x(0, nc.sync, ((0, 2),))
        issue_x(1, nc.sync, ((0, 2),))
        nc.scalar.dma_start(x16[:, 0:2, :], xT16[:, 0:2, :])
        nc.sync.dma_start(x8[:, 2:4, :], xT8h[:, 2:4, :])
        nc.scalar.dma_start(ax[0][:, 2:8], wp1a[0, :, 2:8])
        nc.scalar.dma_start(bx[0][:, 2:8], wp1b[0, :, 2:8])
        nc.sync.dma_start(x16[:, 2:4, :], xT16[:, 2:4, :])
        nc.scalar.dma_start(ax[1][:, 2:8], wp1a[1, :, 2:8])
        nc.scalar.dma_start(bx[1][:, 2:8], wp1b[1, :, 2:8])
        nc.scalar.dma_start(x16[:, 4:6, :], xT16[:, 4:6, :])
        nc.sync.dma_start(x16[:, 6:8, :], xT16[:, 6:8, :])
        nc.scalar.dma_start(ax[0][:, 8:16], wp1a[0, :, 8:16])
        nc.scalar.dma_start(bx[0][:, 8:16], wp1b[0, :, 8:16])
        nc.sync.dma_start(x16[:, 10:12, :], xT16[:, 10:12, :])
        nc.scalar.dma_start(x16[:, 8:10, :], xT16[:, 8:10, :])
        nc.scalar.dma_start(ax[1][:, 8:16], wp1a[1, :, 8:16])
        nc.scalar.dma_start(bx[1][:, 8:16], wp1b[1, :, 8:16])
        nc.scalar.dma_start(x16[:, 12:14, :], xT16[:, 12:14, :])
        nc.sync.dma_start(x16[:, 14:16, :], xT16[:, 14:16, :])
        nc.sync.dma_start(c8[:, 0:8, :], cT8[:, 0:8, :])
        nc.sync.dma_start(c8[:, 8:16, :], cT8[:, 8:16, :])
        issue_h(0, nc.scalar, split=True)
        issue_h(1, nc.scalar)
        issue_x(2, nc.sync)
        issue_h(2, nc.sync)
        # gpsimd (slow software queue): late-needed bulk
        for lo, hi in ((0, 4), (4, 8), (8, 12), (12, 16)):
            nc.gpsimd.dma_start(h16[:, lo:hi, :], hT16[:, lo:hi, :])
        issue_c0(0, nc.gpsimd)
        issue_c0(1, nc.gpsimd)
        nc.gpsimd.dma_start(bias_sb[:], bias[:])
        # derive the remaining fp8 forms on the vector engine
        for lo, hi in ((4, 6), (6, 8), (8, 10), (10, 12), (12, 14), (14, 16)):
            nc.vector.tensor_copy(out=x8[:, lo:hi, :], in_=x16[:, lo:hi, :])
        for lo, hi in ((0, 4), (4, 8), (8, 12), (12, 16)):
            nc.vector.tensor_copy(out=h8[:, lo:hi, :], in_=h16[:, lo:hi, :])

        # ---- phase 1 ----
        TOT = {"i": 2 * NPAIR, "f": 3 * NPAIR, "g": 2 * nk}
        rot = (nc.scalar, nc.sync, nc.gpsimd)
        ps = {}
        idx = {}

        def mm(gate, m, lhsT, rhs, perf_mode=None):
            st = idx[(gate, m)] == 0
            sp = idx[(gate, m)] == TOT[gate] - 1
            nc.tensor.matmul(ps[(gate, m)][:], lhsT=lhsT, rhs=rhs,
                             start=st, stop=sp, perf_mode=perf_mode)
            idx[(gate, m)] += 1

        def open_ps(m):
            for g in ("i", "f", "g"):
                ps[(g, m)] = ppool.tile([p, b], f32, tag="ps",
                                        name=f"ps_{g}{m}")
                idx[(g, m)] = 0

        def emit_xj(m, j):
            mm("i", m, ax[m][:, 2 * j], x8[:, 2 * j:2 * j + 2], DRS)
            mm("f", m, ax[m][:, 2 * j + 1], x8[:, 2 * j:2 * j + 2], DRS)
            mm("g", m, bx[m][:, 2 * j], x16[:, 2 * j])
            mm("g", m, bx[m][:, 2 * j + 1], x16[:, 2 * j + 1])

        def emit_h(m):
            for j in range(NPAIR):
                mm("i", m, ah[m][:, 3 * j], h8[:, 2 * j:2 * j + 2], DRS)
                mm("f", m, ah[m][:, 3 * j + 1], h8[:, 2 * j:2 * j + 2], DRS)
                mm("f", m, ah[m][:, 3 * j + 2], c8[:, 2 * j:2 * j + 2], DRS)
                mm("g", m, bh[m][:, 2 * j], h16[:, 2 * j])
                mm("g", m, bh[m][:, 2 * j + 1], h16[:, 2 * j + 1])

        def acts_for(m):
            i_act = tpool.tile([p, b], f32, tag="i_act")
            nc.scalar.activation(i_act[:], ps[("i", m)][:], Sig,
                                 bias=bias_sb[:, m, 0:1], scale=INV_S)
            f_act = tpool.tile([p, b], f32, tag="f_act")
            nc.scalar.activation(f_act[:], ps[("f", m)][:], Sig,
                                 bias=bias_sb[:, m, 1:2], scale=INV_S)
            g_act = tpool.tile([p, b], f32, tag="g_act")
            nc.scalar.activation(g_act[:], ps[("g", m)][:], Tanh,
                                 bias=bias_sb[:, m, 2:3], scale=INV_S)
            t1 = tpool.tile([p, b], f32, tag="t1")
            nc.vector.tensor_mul(t1[:], f_act[:], c0_t[m][:])
            nc.vector.tensor_mul(i_act[:], i_act[:], g_act[:])
            c1_m = c1b[:, m, :]
            nc.vector.tensor_add(c1_m, t1[:], i_act[:])
            nc.scalar.activation(c18[:, m, :], c1_m, Copy, scale=SA)
            nc.sync.dma_start(c1T[:, m, :], c1_m)

        # Delta=2 ramp block: x-parts of m0+m1 interleaved per k-chunk
        open_ps(0)
        open_ps(1)
        for j in range(NPAIR):
            emit_xj(0, j)
            emit_xj(1, j)

        # steady iterations: h(k-1) + acts, then x(k+1)
        for k in range(1, mt + 1):
            if k + 2 < mt:
                alloc_x(k + 2)
                issue_x(k + 2, rot[k % 3])
            if 3 <= k + 1 < mt:
                alloc_h(k + 1)
                issue_h(k + 1, rot[k % 3])
            if k + 1 < mt:
                issue_c0(k + 1, nc.gpsimd)
            if k == mt - 1:
                alloc_p2(0); issue_p2(0, nc.scalar)
                alloc_p2(1); issue_p2(1, nc.scalar)
            emit_h(k - 1)
            acts_for(k - 1)
            if k + 1 < mt:
                open_ps(k + 1)
                for j in range(NPAIR):
                    emit_xj(k + 1, j)

        # ---- phase 2: o gate + h1 ----
        for m in range(mt):
            if m + 2 < mt:
                alloc_p2(m + 2)
                issue_p2(m + 2, nc.sync if m % 2 else nc.scalar)
            ps_o = ppool.tile([p, b], f32, tag="ps", name=f"ps_o{m}")
            # tanh(c1) first: scalar computes it while the matmuls run
            tc1 = tpool.tile([p, b], bf16, tag="tc1")
            nc.scalar.activation(tc1[:], c1b[:, m, :], Tanh)
            for j in range(NPAIR):
                st = j == 0
                sp = j == NPAIR - 1
                nc.tensor.matmul(ps_o[:], lhsT=slab2[m][:, 3 * j],
                                 rhs=x8[:, 2 * j:2 * j + 2],
                                 start=st, stop=False, perf_mode=DRS)
                nc.tensor.matmul(ps_o[:], lhsT=slab2[m][:, 3 * j + 1],
                                 rhs=h8[:, 2 * j:2 * j + 2],
                                 start=False, stop=False, perf_mode=DRS)
                nc.tensor.matmul(ps_o[:], lhsT=slab2[m][:, 3 * j + 2],
                                 rhs=c18[:, 2 * j:2 * j + 2, :],
                                 start=False, stop=sp, perf_mode=DRS)

            oh = tpool.tile([p, 2, b], bf16, tag="oh")
            nc.scalar.activation(oh[:, 0, :], ps_o[:], Sig,
                                 bias=bias_sb[:, m, 3:4], scale=INV_S)
            nc.vector.tensor_mul(oh[:, 1, :], oh[:, 0, :], tc1[:])
            nc.sync.dma_start(ohT[:, m], oh[:])

    nc.compile()
    return nc


_NC = None


def _get_nc():
    global _NC
    if _NC is None:
        _NC = _build(P, NK, MT, B)
    return _NC


# ---------------- host-side packing ----------------

def _pack_actT(a, dtype, scale=1.0):
    """(b, d) -> (128, d//128, b) with [ki, ko, b] = a[b, ko*128+ki]."""
    b, d = a.shape
    at = np.ascontiguousarray(a.T.reshape(d // P, P, b).transpose(1, 0, 2))
    if scale != 1.0:
        at = np.clip(at * scale, -240.0, 240.0)
    return at.astype(dtype, copy=False)


def _pack_w(W, dtype, scale):
    """(H, K) -> (H//128, 128, 16, 128) with [mt, ki, ko, m] = W[mt*128+m, ko*128+ki]."""
    H, K = W.shape
    r = (W * scale).reshape(H // P, P, K // P, P).transpose(0, 3, 2, 1)
    return np.ascontiguousarray(r).astype(dtype)


def _pack_w8_swi(W, scale):
    """fp8 weights in DoubleRowSwInterleave layout: per k-tile pair (A, B),
    each SBUF row holds [A127, B127, A126, B126, ..., A0, B0].
    (H, K) -> (H//128, 128, 8, 256)."""
    r = _pack_w(W, F8, scale)                    # [mt, ki, kt, m]
    A = r[:, :, 0::2, ::-1]                      # [mt, ki, pair, m-reversed]
    Bm = r[:, :, 1::2, ::-1]
    return np.ascontiguousarray(
        np.stack([A, Bm], axis=-1).reshape(A.shape[0], P, A.shape[2], 2 * P))


def _interleave(mats):
    """list of [mt, p, n, w] -> [mt, p, n*len, w] with columns round-robin."""
    s = np.stack(mats, axis=3)                   # [mt, p, n, L, w]
    return np.ascontiguousarray(
        s.reshape(s.shape[0], s.shape[1], -1, s.shape[4]))


def _unpack_out(o):
    """(128, mt, b) [p, m, b] -> (b, mt*128) fp32."""
    p, m, b = o.shape
    return np.ascontiguousarray(
        o.transpose(2, 1, 0).reshape(b, m * p).astype(np.float32))


def kernel(x, h0, c0,
           W_ii, b_ii, W_hi, b_hi, W_if_, b_if_, W_hf, b_hf, W_cf, b_cf,
           W_ic, b_ic, W_hc, b_hc, W_io, b_io, W_ho, b_ho, W_co, b_co,
           _trace=False):
    from concourse.bass_utils import run_bass_kernel_spmd

    nc = _get_nc()

    x = np.asarray(x, dtype=np.float32)
    h0 = np.asarray(h0, dtype=np.float32)
    c0 = np.asarray(c0, dtype=np.float32)
    Ws = {n: np.asarray(a, dtype=np.float32)
          for n, a in (("ii", W_ii), ("hi", W_hi), ("if_", W_if_),
                       ("hf", W_hf), ("cf", W_cf), ("ic", W_ic),
                       ("hc", W_hc), ("io", W_io), ("ho", W_ho),
                       ("co", W_co))}
    (b_ii, b_hi, b_if_, b_hf, b_cf, b_ic, b_hc, b_io, b_ho, b_co) = [
        np.asarray(a, dtype=np.float32)
        for a in (b_ii, b_hi, b_if_, b_hf, b_cf, b_ic, b_hc, b_io, b_ho, b_co)
    ]

    # combined per-gate biases, packed [p, mt, gate]
    bias = np.stack(
        [
            (b_ii + b_hi).reshape(MT, P).T,
            (b_if_ + b_hf + b_cf).reshape(MT, P).T,
            (b_ic + b_hc).reshape(MT, P).T,
            (b_io + b_ho + b_co).reshape(MT, P).T,
        ],
        axis=2,
    ).astype(np.float32)

    w8 = {n: _pack_w8_swi(Ws[n], SW)
          for n in ("ii", "if_", "hi", "hf", "cf", "io", "ho", "co")}
    wp1a = np.concatenate(
        [_interleave([w8["ii"], w8["if_"]]),
         _interleave([w8["hi"], w8["hf"], w8["cf"]])], axis=2)
    wp1b = np.concatenate(
        [_pack_w(Ws["ic"], BF, SW), _pack_w(Ws["hc"], BF, SW)], axis=2)
    wp2a = _interleave([w8["io"], w8["ho"], w8["co"]])

    in_maps = []
    for core in range(N_CORES):
        s = slice(core * B, (core + 1) * B)
        xT16 = _pack_actT(x[s], BF, SA)
        in_maps.append({
            "xT16": xT16,
            # fp8 head for k-tiles 0..3: first matmuls skip the on-device
            # cast; matches the device cast (bf16 -> fp8 double rounding)
            "xT8h": np.ascontiguousarray(xT16[:, 0:4, :]).astype(F8),
            "hT16": _pack_actT(h0[s], BF, SA),
            "cT8": _pack_actT(c0[s], F8, SA),
            "c0T": _pack_actT(c0[s], np.float32),
            "bias": bias,
            "wp1a": wp1a, "wp1b": wp1b, "wp2a": wp2a,
        })

    res = run_bass_kernel_spmd(nc, in_maps, list(range(N_CORES)), trace=_trace)

    o_g = np.empty((BATCH, HID), np.float32)
    h1 = np.empty((BATCH, HID), np.float32)
    c1 = np.empty((BATCH, HID), np.float32)
    for core in range(N_CORES):
        s = slice(core * B, (core + 1) * B)
        oh = res.results[core]["ohT"]
        o_g[s] = _unpack_out(oh[:, :, 0, :])
        h1[s] = _unpack_out(oh[:, :, 1, :])
        c1[s] = _unpack_out(res.results[core]["c1T"])
    out = (o_g, h1, c1)
    if _trace:
        return out, res
    return out
